# revision 9
# baseline (speedup 1.0000x reference)
"""Trainium2 Bass kernel for 16-group CustomGroupedConv2D.

Problem (hardcoded): x (16, 256, 128, 128) f32, W (512, 16, 3, 3) f32,
b (512,) f32, groups=16, 3x3, stride 1, pad 1 -> y (16, 512, 128, 128) f32.

Sharding: data-parallel over batch, 2 images per core on 8 cores; each core
writes its own output slice (no collectives).

Compute: the 128x128 PE array is a 4x2 grid of 32x64 sub-arrays via
tile_position; each holds a block-diagonal group PAIR (K=32: two groups' 16
cins; M=64: their couts). The 9 conv taps are 9 full-width accumulating
passes (PSUM start/stop) whose shifted windows are pure AP offsets into a
zero-padded 132-pitch SBUF image buffer.

Phased I/O (measured: HBM->SBUF load DMAs overlapping PE execution tax the
PE ~2x, while SBUF->HBM stores are free, and serializing the loads costs
less than the overlap tax):
- x is cast bf16 on the host; per half, two fully contiguous 2.13MB loads
  land in a flat staging buffer. The loads are triggered from the Act/SP
  queues, so engine program order gates them behind the previous half's
  evacuations/stores: they transfer while the PE is between halves instead
  of taxing it mid-compute.
- VectorE re-lays staging into the padded 132-pitch buffer (pads and halo
  slabs are zeroed once at startup and never rewritten).
- y is written bf16 (host upcasts; ~1e-3 extra rounding, tolerance is
  2e-2) with stores batched 4 windows per DMA: 128 stores of 4KB/line.

Bias is fused into the PSUM->SBUF evacuation (ScalarE even banks, VectorE
odd) along with the f32->bf16 cast. Each window's 4 PSUM banks are shared
by the (r, s=0/1) PE tiles (partitions 0:64/64:128); hardware clears
has_written per partition-range on each tile's first start=True matmul.
"""

import numpy as np

N_CORES = 8
N, CIN, H, W_IMG = 16, 256, 128, 128
COUT, KH, KW = 512, 3, 3
GROUPS = 16
CPG = CIN // GROUPS  # 16 cins per group
MPG = COUT // GROUPS  # 32 couts per group
N_PER_CORE = N // N_CORES  # 2 images
SLABS = 66  # padded row slabs per half (65 rows + 1 halo/zero)
WPAD = 132  # padded row pitch (col 0 and 129 are the zero pads)
WIN_ROWS = 4  # output rows per window (N = 4*128 = 512)
WINS = 16  # windows per half
SW_WINS = 4  # windows per store super-window (16 output rows per store DMA)

_CACHE = {}


def _bank_groups(r):
    """Groups whose couts live in psum bank r, in col-strip order."""
    return [2 * r, 2 * r + 1, 2 * r + 8, 2 * r + 9]


def _prep_weights(W):
    # W: (COUT, CPG, 3, 3) -> W_prep [128, 9, 2, 64] bf16, block-diagonal
    # group pairs: partition 32r+i, tap t=3*dy+dx, slot s holds the [32, 64]
    # lhsT for the pair (g0, g1) = (8s+2r, 8s+2r+1).
    import ml_dtypes

    Wp = np.zeros((128, KH * KW, 2, 2 * MPG), np.float32)
    for r in range(4):
        for s in range(2):
            for half in range(2):
                g = 8 * s + 2 * r + half
                blk = W[g * MPG : (g + 1) * MPG]  # (32, 16, 3, 3)
                lhsT = np.transpose(blk, (1, 2, 3, 0)).reshape(CPG, KH * KW, MPG)
                Wp[
                    32 * r + 16 * half : 32 * r + 16 * (half + 1),
                    :,
                    s,
                    MPG * half : MPG * (half + 1),
                ] = lhsT
    return Wp.astype(ml_dtypes.bfloat16)


def _prep_bias(b):
    # b: (COUT,) -> b_prep [128, 4]; partition 32j+m, col r = b[G(r,j)*32+m]
    br = b.reshape(GROUPS, MPG)
    bp = np.zeros((128, 4), np.float32)
    for r in range(4):
        for j, g in enumerate(_bank_groups(r)):
            bp[32 * j : 32 * j + 32, r] = br[g]
    return bp


def _build_program(reps=1, mode="full"):
    # mode: "full" | "dma_only" (loads+copies+stores, no matmul/evac) |
    # "no_store" (everything but y DMAs) — timing decomposition only.
    import concourse.bacc as bacc
    import concourse.mybir as mybir
    import concourse.tile as tile
    from contextlib import nullcontext

    f32 = mybir.dt.float32
    bf16 = mybir.dt.bfloat16
    ACT_IDENT = mybir.ActivationFunctionType.Identity

    nc = bacc.Bacc(
        "TRN2", target_bir_lowering=False, debug=False, num_devices=N_CORES
    )
    x_d = nc.dram_tensor("x", [N_PER_CORE, CIN, H, W_IMG], bf16, kind="ExternalInput")
    w_d = nc.dram_tensor("wp", [128, 9, 2, 2 * MPG], bf16, kind="ExternalInput")
    b_d = nc.dram_tensor("bp", [128, 4], f32, kind="ExternalInput")
    y_d = nc.dram_tensor(
        "y", [N_PER_CORE, COUT, H, W_IMG], bf16, kind="ExternalOutput"
    )

    with tile.TileContext(nc) as tc:
        with (
            tc.tile_pool(name="wpool", bufs=1) as wpool,
            tc.tile_pool(name="xstgp", bufs=2) as xstgp,
            tc.tile_pool(name="ppool", bufs=8, space="PSUM") as ppool,
            tc.tile_pool(name="spool", bufs=2) as spool,
        ):
            w_sb = wpool.tile([128, 9, 2, 2 * MPG], bf16, tag="w")
            nc.sync.dma_start(w_sb[:], w_d[:])
            b_sb = wpool.tile([128, 4], f32, tag="b")
            nc.sync.dma_start(b_sb[:], b_d[:])
            # one padded image buffer per half-parity; pads and halo slabs
            # are zeroed here once and never rewritten
            xv = [
                wpool.tile([128, 2, SLABS, WPAD], bf16, tag=f"xv{p}", name="xv")
                for p in range(2)
            ]
            for p in range(2):
                nc.gpsimd.memset(xv[p][:], 0.0)
            static_stg = None
            if mode == "dma_only":
                static_stg = wpool.tile(
                    [128, SW_WINS * WIN_ROWS, W_IMG], bf16, tag="sstg"
                )
                nc.gpsimd.memset(static_stg[:], 0.5)

            # reps>1 repeats the whole computation on-device (timing only)
            rep_ctx = tc.For_i(0, reps, 1) if reps > 1 else nullcontext()
            with rep_ctx:
              for n in range(N_PER_CORE):
                  for hf in range(2):
                      # contiguous staging loads; Act/SP-queue triggers gate
                      # them behind the previous half's evacs/stores so the
                      # transfers don't overlap (and tax) PE compute
                      xstg = xstgp.tile([128, 2, 65, W_IMG], bf16, tag="xstg")
                      row0 = 0 if hf == 0 else 63
                      nc.scalar.dma_start(
                          xstg[:, 0], x_d[n, 0:128, row0 : row0 + 65, :]
                      )
                      nc.sync.dma_start(
                          xstg[:, 1], x_d[n, 128:256, row0 : row0 + 65, :]
                      )
                      # VectorE re-layout into the padded buffer
                      # (hf0: rows 0..64 -> slabs 1..65; hf1: rows 63..127 ->
                      # slabs 0..64)
                      slab0 = 1 if hf == 0 else 0
                      xb = xv[hf]
                      for s in range(2):
                          nc.vector.tensor_copy(
                              xb[:, s, slab0 : slab0 + 65, 1 : 1 + W_IMG],
                              xstg[:, s],
                          )
                      for sw in range(WINS // SW_WINS):
                          stg = [
                              spool.tile(
                                  [128, SW_WINS * WIN_ROWS, W_IMG],
                                  bf16,
                                  tag=f"stg{r}",
                                  name="stg",
                              )
                              for r in range(4)
                          ]
                          if mode == "dma_only":
                              out_row0 = 64 * hf + SW_WINS * WIN_ROWS * sw
                              for r in range(4):
                                  for s, co0 in ((0, 64 * r), (1, 256 + 64 * r)):
                                      nc.sync.dma_start(
                                          y_d[
                                              n,
                                              co0 : co0 + 64,
                                              out_row0 : out_row0
                                              + SW_WINS * WIN_ROWS,
                                              :,
                                          ],
                                          static_stg[64 * s : 64 * s + 64, :, :],
                                      )
                              continue
                          for wl in range(SW_WINS):
                              w = SW_WINS * sw + wl
                              ps = [
                                  ppool.tile(
                                      [128, WIN_ROWS, W_IMG], f32, tag="ps", name="ps"
                                  )
                                  for _ in range(4)
                              ]
                              for t in range(9):
                                  dy, dx = t // 3, t % 3
                                  for r in range(4):
                                      for s in range(2):
                                          nc.tensor.matmul(
                                              ps[r][64 * s : 64 * s + 64, :, :],
                                              w_sb[32 * r : 32 * r + 32, t, s, :],
                                              xb[
                                                  32 * r : 32 * r + 32,
                                                  s,
                                                  WIN_ROWS * w + dy : WIN_ROWS * w
                                                  + dy
                                                  + WIN_ROWS,
                                                  dx : dx + W_IMG,
                                              ],
                                              start=(t == 0),
                                              stop=(t == 8),
                                              tile_position=(32 * r, 64 * s),
                                              skip_group_check=True,
                                          )
                              # evacuate into this window's quarter of the
                              # super-window staging tiles, fusing bias and
                              # the f32->bf16 cast; ScalarE/VectorE split
                              for r in range(4):
                                  dst = stg[r][
                                      :, WIN_ROWS * wl : WIN_ROWS * (wl + 1), :
                                  ]
                                  if r % 2 == 0:
                                      nc.scalar.activation(
                                          dst,
                                          ps[r][:],
                                          ACT_IDENT,
                                          bias=b_sb[:, r : r + 1],
                                      )
                                  else:
                                      nc.vector.tensor_scalar_add(
                                          dst,
                                          ps[r][:],
                                          b_sb[:, r : r + 1],
                                      )
                          # one store DMA per (r, s) per super-window:
                          # 16 output rows, 4 KB/partition-line
                          out_row0 = 64 * hf + SW_WINS * WIN_ROWS * sw
                          if mode == "no_store":
                              continue
                          for r in range(4):
                              # couts: partitions 0:64 -> 64r..64r+64 (s=0),
                              # partitions 64:128 -> 256+64r.. (s=1)
                              for s, co0 in ((0, 64 * r), (1, 256 + 64 * r)):
                                  nc.sync.dma_start(
                                      y_d[
                                          n,
                                          co0 : co0 + 64,
                                          out_row0 : out_row0 + SW_WINS * WIN_ROWS,
                                          :,
                                      ],
                                      stg[r][64 * s : 64 * s + 64, :, :],
                                  )

    nc.compile()
    return nc


def _get_program(reps=1):
    key = ("nc", reps)
    if key not in _CACHE:
        _CACHE[key] = _build_program(reps)
    return _CACHE[key]


def make_in_maps(x, W, b):
    import ml_dtypes

    Wp = _prep_weights(np.asarray(W, dtype=np.float32))
    bp = _prep_bias(np.asarray(b, dtype=np.float32))
    x_bf = np.ascontiguousarray(
        np.asarray(x, dtype=np.float32).astype(ml_dtypes.bfloat16)
    )
    return [
        {
            "x": x_bf[i * N_PER_CORE : (i + 1) * N_PER_CORE],
            "wp": Wp,
            "bp": bp,
        }
        for i in range(N_CORES)
    ]


def kernel(x, W, b):
    from concourse.bass_utils import run_bass_kernel_spmd

    nc = _get_program()
    in_maps = make_in_maps(x, W, b)
    res = run_bass_kernel_spmd(nc, in_maps, list(range(N_CORES)))
    out = np.concatenate([res.results[i]["y"] for i in range(N_CORES)], axis=0)
    return out.astype(np.float32)
